# revision 71
# baseline (speedup 1.0000x reference)
"""AudioCondTransformerEncoderLayer on 8 Trainium2 NeuronCores.

v2 strategy (TM=TA=512, B=32, D=1024, H=16, DFF=4096, 4 batch elems/core):
  - Data-parallel over batch across 8 cores; per-core per-b pipeline with
    cross-b overlap via pooled buffers (bf16 tiles halve SBUF so tensors can
    be multi-buffered across batch elements).
  - All matmul operands bf16 (PSUM accumulates f32); residual stream bf16.
  - Cross-attention is BANDED: the temporal bias -(q-k)^2/32 kills everything
    beyond |q-k|~54, so tk-chunk c only scores against the contiguous tq
    range [128(c-1), 128(c+2)) (clamped); bias tiles are added in PSUM via an
    identity-weight matmul, padded with -1e9 so out-of-band exp underflows
    to exactly 0.
  - Softmax denominator via a ones column appended to v (65-col attnV);
    reciprocal on DVE, broadcast across partitions with a rank-1 PE outer
    product through a dedicated 1-bank psum pool, ACT-copied out.
  - LayerNorm: partition sums via ones-column bf16 matmuls, variance via one
    fused scalar_tensor_tensor, sqrt on ACT, recip on DVE; mean/invstd
    broadcast via PE outer products (mean early, hidden under the sumsq
    loop); apply = two DVE tensor ops when gain==1/bias==0 (this problem),
    general 3-op fallback otherwise.
  - Projection PSUM drains on ACT (Identity+bias) so DVE backlog never
    stalls the PE through the 2-slot projection psum pool; k-accumulation
    order rotated per output chunk so the last-needed input differs.
  - CA k/v projections emitted before the SA out-proj (audio-only deps) to
    feed the PE during the SA attention tail.
  - FFN: lin1+Gelu -> hT bf16 quarters -> lin2, residual fused via stt.
  - PSUM budget: proj 3 + LN-sums 1 + scores/attnV-op 3 + broadcast 1 = 8.
  - Avoid nc.gpsimd elementwise ops: ~14us/op on this HW stack.
"""

import numpy as np

# ---------------------------------------------------------------------------
# Problem constants
# ---------------------------------------------------------------------------
D = 1024
H = 16
HD = 64
TM = 512
TA = 512
B = 32
DFF = 4096
NCORES = 8
BPC = B // NCORES          # batch elems per core
SIGMA = 4.0
BW = 2.0
LN_EPS = 1e-5
KD = D // 128              # 8 d-chunks
KF = DFF // 128            # 32 ff-chunks
TCH = TM // 128            # 4 token chunks

# banded CA: tk-chunk c attends to queries in [TQR[c][0], +TQR[c][1])
TQR = [(0, 256), (0, 384), (128, 384), (256, 256)]
BBW = 384                  # padded bias tile width

_CACHE = {}


# ---------------------------------------------------------------------------
# Walrus workaround: this container's walrus build rejects >1 sync-wait per
# instruction. Split excess waits onto preceding same-engine NOPs, and move
# the tail drain's waits onto SP NOPs.
# ---------------------------------------------------------------------------
def _install_patches():
    if _CACHE.get("patched"):
        return
    import concourse.mybir as mybir
    import concourse.tile as tile
    import concourse.tile_utils as tile_utils
    from concourse.vector_clock import ScopedClock

    tile_utils.max_sbuf_usage = 208 * 1024

    _orig_commit = tile.TileContext._commit_instruction

    def _split_commit(self, inst, lazy_reg_writes=True):
        si = inst.sync_info
        if (
            si is not None
            and len(si.on_wait) > 1
            and inst.engine != mybir.EngineType.Unassigned
        ):
            waits = list(si.on_wait)
            inst.sync_info = mybir.SyncInfo(
                on_wait=waits[:1], on_update=list(si.on_update)
            )
            for w in waits[1:]:
                nop = mybir.InstNoOp(
                    name=self.nc.get_next_instruction_name(),
                    ins=[],
                    outs=[],
                    engine=inst.engine,
                    sync_info=mybir.SyncInfo(on_wait=[w], on_update=[]),
                )
                nop.debug = inst.debug
                _orig_commit(self, nop, lazy_reg_writes=False)
        return _orig_commit(self, inst, lazy_reg_writes=lazy_reg_writes)

    tile.TileContext._commit_instruction = _split_commit

    def _patched_drain_and_barrier(self, tick_clock, wait_clock):
        carrier = self.nc.sync.nop(nofuse=True)
        wait_clock.add_sem_waits(
            carrier.ins, ScopedClock({None: tick_clock.global_clock})
        )
        si = carrier.ins.sync_info
        if si is not None and len(si.on_wait) > 1:
            waits = list(si.on_wait)
            carrier.ins.sync_info = mybir.SyncInfo(
                on_wait=waits[:1], on_update=list(si.on_update)
            )
            for w in waits[1:]:
                extra = self.nc.sync.nop(nofuse=True)
                extra.ins.sync_info = mybir.SyncInfo(on_wait=[w], on_update=[])
        self.nc.sync.drain()
        self.nc.all_engine_barrier()
        popped = self.nc._tile_sem_poison_stack.pop()
        assert popped is self._sem_poison
        self.nc.clear_and_free_semaphores(list(self.sems.allocated().values()))
        self.nc.all_engine_barrier()

    tile.TileContext._drain_and_barrier = _patched_drain_and_barrier
    _CACHE["patched"] = True


# ---------------------------------------------------------------------------
# Device module
# ---------------------------------------------------------------------------
def _build_module(ln_affine_identity):
    from contextlib import ExitStack

    import concourse.bass as bass
    import concourse.mybir as mybir
    import concourse.tile as tile

    f32 = mybir.dt.float32
    f32r = mybir.dt.float32r
    bf16 = mybir.dt.bfloat16
    AF = mybir.ActivationFunctionType
    OP = mybir.AluOpType

    nc = bass.Bass()

    def din(name, shape, dt=bf16):
        return nc.dram_tensor(name, shape, dt, kind="ExternalInput")

    xin = din("xin", (BPC, KD, 128, TM))
    ain = din("ain", (BPC, KD, 128, TA))
    wqk_sa = din("wqk_sa", (16, 128, KD, 128))
    wv_sa = din("wv_sa", (4, 128, KD, 256))
    wo_sa = din("wo_sa", (8, 128, KD, 128))
    wqk_ca = din("wqk_ca", (16, 128, KD, 128))
    wv_ca = din("wv_ca", (4, 128, KD, 256))
    wo_ca = din("wo_ca", (8, 128, KD, 128))
    w1 = din("w1", (KF, 128, KD, 128))
    w2 = din("w2", (8, 2, 128, KF // 2, 128))
    # all per-partition bias/gain vectors packed into one tensor:
    # [bqk_sa(16) bo_sa(8) bqk_ca(16) bo_ca(8) b1(32) b2(8)
    #  n1g n1b ncg ncb n2g n2b (6x8)] = 136 cols
    ppb = din("ppb", (128, 136), f32)
    biasb = din("biasb", (4, 128, BBW), f32)
    ident = din("ident", (128, 128), f32)
    onescol = din("onescol", (128, 1), f32)
    onesrow = din("onesrow", (1, 128), f32)

    out = nc.dram_tensor("out", (BPC, KD, 128, TM), bf16, kind="ExternalOutput")

    with tile.TileContext(nc) as tc, ExitStack() as ctx:
        cpool = ctx.enter_context(tc.tile_pool(name="consts", bufs=1))
        # one big-activation pool: 8.32KB slots shared by every [128,KD,512]
        # bf16 stream tensor plus v ([128,4,1040]) and hT quarters
        actp = ctx.enter_context(tc.tile_pool(name="acts", bufs=13))
        expp = ctx.enter_context(tc.tile_pool(name="expS", bufs=8))
        wp = ctx.enter_context(tc.tile_pool(name="wstream", bufs=8))
        wvp = ctx.enter_context(tc.tile_pool(name="wvstream", bufs=4))
        w2p = ctx.enter_context(tc.tile_pool(name="w2stream", bufs=2))
        smp = ctx.enter_context(tc.tile_pool(name="small", bufs=3))
        smrp = ctx.enter_context(tc.tile_pool(name="smallr", bufs=4))
        bcp = ctx.enter_context(tc.tile_pool(name="bcast", bufs=2))
        tmpp = ctx.enter_context(tc.tile_pool(name="tmp", bufs=3))
        sqp = ctx.enter_context(tc.tile_pool(name="sq", bufs=4))
        scrp = ctx.enter_context(tc.tile_pool(name="scratch", bufs=2))
        # psum: psa 3 + pss(ln sums) 1 + scorep(scores+op) 3 + bcb 1 = 8
        psa = ctx.enter_context(tc.tile_pool(name="psa", bufs=3, space="PSUM"))
        pss = ctx.enter_context(tc.tile_pool(name="pss", bufs=1, space="PSUM"))
        scorep = ctx.enter_context(tc.tile_pool(name="scorep", bufs=3, space="PSUM"))
        bcb = ctx.enter_context(tc.tile_pool(name="bcb", bufs=1, space="PSUM"))

        # --- constants -----------------------------------------------------
        ident_r = cpool.tile([128, 128], f32r, name="ident_r")
        nc.sync.dma_start(ident_r[:], ident[:, :].bitcast(f32r))
        ones_cb = cpool.tile([128, 1], bf16, name="ones_cb")
        ones_c32 = cpool.tile([128, 1], f32, name="ones_c32")
        nc.sync.dma_start(ones_c32[:], onescol[:, :])
        nc.vector.tensor_copy(ones_cb[:], ones_c32[:])
        ones_r = cpool.tile([1, 128], f32r, name="ones_r")
        nc.sync.dma_start(ones_r[:], onesrow[:, :].bitcast(f32r))
        biasb_r = cpool.tile([128, 4, BBW], f32r, name="biasb_r")
        nc.sync.dma_start(
            biasb_r[:], biasb[:, :, :].rearrange("c p t -> p c t").bitcast(f32r))
        eps_t = cpool.tile([1, 1], f32, name="eps_t")
        nc.vector.memset(eps_t[:], LN_EPS)

        ppb_t = cpool.tile([128, 136], f32, name="ppb_t")
        nc.sync.dma_start(ppb_t[:], ppb[:, :])
        _off = [0]

        def pp_view(n):
            o = _off[0]
            _off[0] += n
            return ppb_t[:, o:o + n]

        bqk_sa_t = pp_view(16)
        bo_sa_t = pp_view(8)
        bqk_ca_t = pp_view(16)
        bo_ca_t = pp_view(8)
        b1_t = pp_view(KF)
        b2_t = pp_view(8)
        n1g_t, n1b_t = pp_view(8), pp_view(8)
        ncg_t, ncb_t = pp_view(8), pp_view(8)
        n2g_t, n2b_t = pp_view(8), pp_view(8)

        def a8(name):
            return actp.tile([128, KD, TM], bf16, tag="a8", name=name)

        # --- helpers -------------------------------------------------------
        def ln(y, g_t, b_t, dst):
            """LayerNorm over the partition (feature) axis of y [128,KD,T]
            bf16; dst bf16."""
            ps_s = pss.tile([1, TM], f32, tag="sps", name="ps_s")
            for k in range(KD):
                nc.tensor.matmul(ps_s[:], ones_cb[:], y[:, k],
                                 start=(k == 0), stop=(k == KD - 1))
            mi = smp.tile([1, 2, TM], f32r, tag="mi", name="mi")
            with nc.allow_low_precision(reason="ln mean f32r for bcast mm"):
                nc.scalar.mul(mi[:, 0], ps_s[:], 1.0 / D)
            miB = bcp.tile([128, 2, TM], f32, tag="bcl", name="miB")
            bmu = bcb.tile([128, TM], f32, tag="bc", name="bmu")
            nc.tensor.matmul(bmu[:], ones_r[:], mi[:, 0], start=True, stop=True)
            nc.scalar.copy(miB[:, 0], bmu[:])
            ps_q = pss.tile([1, TM], f32, tag="sps", name="ps_q")
            for k in range(KD):
                sq = sqp.tile([128, TM], bf16, tag="sq", name="sq")
                if k % 2 == 0:
                    nc.vector.tensor_tensor(sq[:], y[:, k], y[:, k], OP.mult)
                else:
                    nc.scalar.activation(sq[:], y[:, k], AF.Square)
                nc.tensor.matmul(ps_q[:], ones_cb[:], sq[:],
                                 start=(k == 0), stop=(k == KD - 1))
            m2 = smp.tile([1, TM], f32, tag="sm", name="m2")
            nc.vector.tensor_tensor(m2[:], mi[:, 0].bitcast(f32),
                                    mi[:, 0].bitcast(f32), OP.mult)
            var = smp.tile([1, TM], f32, tag="sm", name="var")
            nc.vector.scalar_tensor_tensor(var[:], ps_q[:], 1.0 / D, m2[:],
                                           OP.mult, OP.subtract)
            sd = smp.tile([1, TM], f32, tag="sm", name="sd")
            nc.scalar.activation(sd[:], var[:], AF.Sqrt, bias=eps_t[:])
            with nc.allow_low_precision(reason="ln invstd recip"):
                nc.vector.reciprocal(mi[:, 1], sd[:])
            # invstd broadcast (mean broadcast already issued above, hidden
            # under the sumsq loop)
            biv = bcb.tile([128, TM], f32, tag="bc", name="biv")
            nc.tensor.matmul(biv[:], ones_r[:], mi[:, 1], start=True, stop=True)
            nc.scalar.copy(miB[:, 1], biv[:])
            for k in range(KD):
                eng = nc.vector
                t1 = tmpp.tile([128, TM], bf16, tag="t1", name="t1")
                if ln_affine_identity:
                    eng.tensor_tensor(t1[:], y[:, k], miB[:, 0], OP.subtract)
                    eng.tensor_tensor(dst[:, k], t1[:], miB[:, 1], OP.mult)
                else:
                    nc.vector.tensor_tensor(t1[:], y[:, k], miB[:, 0],
                                            OP.subtract)
                    t2 = tmpp.tile([128, TM], bf16, tag="t1", name="t2")
                    nc.vector.scalar_tensor_tensor(
                        t2[:], t1[:], g_t[:, k:k + 1], miB[:, 1],
                        OP.mult, OP.mult)
                    nc.vector.tensor_scalar_add(dst[:, k], t2[:],
                                                b_t[:, k:k + 1])

        def softmax_av(v, exp_of, hp, onT, banded):
            """attnV + normalize for head pair hp. exp_of[par][cpair] are the
            exp tiles; writes normalized o^T into onT[:, hp] halves."""
            for par in (0, 1):
                h = 2 * hp + par
                ex = exp_of[par]
                op = scorep.tile([65, TM], f32, tag="sc", name="op")
                for c in range(TCH):
                    if banded:
                        lo, w = TQR[c]
                        nc.tensor.matmul(op[:, lo:lo + w],
                                         v[:, c, 65 * h:65 * h + 65],
                                         ex[c][:, 0:w],
                                         start=(c == 0), stop=(c == TCH - 1))
                    else:
                        nc.tensor.matmul(op[:], v[:, c, 65 * h:65 * h + 65],
                                         ex[c][:, :],
                                         start=(c == 0), stop=(c == TCH - 1))
                rr = smrp.tile([1, TM], f32r, tag="smr", name="rr")
                with nc.allow_low_precision(reason="softmax denom recip"):
                    nc.vector.reciprocal(rr[:], op[64:65, :])
                # reciprocal broadcast through the single broadcast psum bank
                bc = bcb.tile([64, TM], f32, tag="bc", name="bct")
                nc.tensor.matmul(bc[:], ones_r[0:1, 0:64], rr[:],
                                 start=True, stop=True)
                bcs = bcp.tile([64, TM], bf16, tag="bcs", name="bcs")
                nc.scalar.copy(bcs[:], bc[:])
                if par == 0:
                    nc.vector.tensor_tensor(onT[0:64, hp, :], op[0:64, :],
                                            bcs[:], OP.mult)
                else:
                    sc = scrp.tile([64, TM], bf16, tag="shift", name="sc")
                    nc.vector.tensor_tensor(sc[:], op[0:64, :], bcs[:], OP.mult)
                    nc.sync.dma_start(onT[64:128, hp, :], sc[:])

        def attention_sa(qT, kT, v, onT):
            for hp in range(H // 2):
                q0 = qT[0:64, hp, :]
                q1 = qT[64:128, hp, :]
                exps = {0: [], 1: []}
                for c in range(TCH):
                    sps0 = scorep.tile([128, TM], f32, tag="sc", name="sps0")
                    sps1 = scorep.tile([128, TM], f32, tag="sc", name="sps1")
                    k0 = kT[0:64, hp, 128 * c:128 * c + 128]
                    k1 = kT[64:128, hp, 128 * c:128 * c + 128]
                    nc.tensor.matmul(sps0[:], k0, q0, start=True, stop=True)
                    nc.tensor.matmul(sps1[:], k1, q1, start=True, stop=True)
                    e0 = expp.tile([128, TM], bf16, tag="e", name="e0")
                    nc.scalar.activation(e0[:], sps0[:], AF.Exp)
                    exps[0].append(e0)
                    e1 = expp.tile([128, TM], bf16, tag="e", name="e1")
                    nc.scalar.activation(e1[:], sps1[:], AF.Exp)
                    exps[1].append(e1)
                softmax_av(v, exps, hp, onT, banded=False)

        def attention_ca(qT, kT, v, onT):
            for hp in range(H // 2):
                exps = {0: [], 1: []}
                for c in range(TCH):
                    lo, w = TQR[c]
                    sps0 = scorep.tile([128, BBW], f32, tag="sc", name="cps0")
                    sps1 = scorep.tile([128, BBW], f32, tag="sc", name="cps1")
                    k0 = kT[0:64, hp, 128 * c:128 * c + 128]
                    k1 = kT[64:128, hp, 128 * c:128 * c + 128]
                    q0 = qT[0:64, hp, lo:lo + w]
                    q1 = qT[64:128, hp, lo:lo + w]
                    nc.tensor.matmul(sps0[:, 0:w], k0, q0,
                                     start=True, stop=False)
                    nc.tensor.matmul(sps0[:, 0:w], ident_r[:],
                                     biasb_r[:, c, 0:w], start=False, stop=True)
                    nc.tensor.matmul(sps1[:, 0:w], k1, q1,
                                     start=True, stop=False)
                    nc.tensor.matmul(sps1[:, 0:w], ident_r[:],
                                     biasb_r[:, c, 0:w], start=False, stop=True)
                    e0 = expp.tile([128, BBW], bf16, tag="e", name="ce0")
                    nc.scalar.activation(e0[:, 0:w], sps0[:, 0:w], AF.Exp)
                    exps[0].append(e0)
                    e1 = expp.tile([128, BBW], bf16, tag="e", name="ce1")
                    nc.scalar.activation(e1[:, 0:w], sps1[:, 0:w], AF.Exp)
                    exps[1].append(e1)
                softmax_av(v, exps, hp, onT, banded=True)

        def qk_proj(wdram, bias_t, srcs, qT, kT, ecs):
            # ec 0..7 -> qT slot ec (src srcs[0]); 8..15 -> kT (src srcs[1])
            for ec in ecs:
                wt = wp.tile([128, KD, 128], bf16, tag="w8", name="wqkt")
                nc.sync.dma_start(wt[:], wdram[ec])
                src = srcs[0] if ec < 8 else srcs[1]
                dst = qT if ec < 8 else kT
                ps = psa.tile([128, TM], f32, tag="mm", name="qkps")
                for j in range(KD):
                    k = (j + ec + 1) % KD
                    nc.tensor.matmul(ps[:], wt[:, k], src[:, k],
                                     start=(j == 0), stop=(j == KD - 1))
                # drain on ACT so DVE backlog can't stall the PE via PSUM
                nc.scalar.activation(dst[:, ec % 8], ps[:], AF.Identity,
                                     bias=bias_t[:, ec:ec + 1])

        def v_proj(wdram, srcT, vdst):
            # ones columns for the softmax-denominator trick
            nc.vector.memset(vdst[:, :, 64::65], 1.0)
            # stationary = activation chunk, shared across all 4 qt blocks:
            # one LDWEIGHTS per (tch, k) instead of per (qt, tch, k)
            wvts = []
            for qt in range(4):
                wvt = wvp.tile([128, KD, 256], bf16, tag="wv", name="wvt")
                nc.sync.dma_start(wvt[:], wdram[qt])
                wvts.append(wvt)
            for tch in range(TCH):
                ps0 = psa.tile([128, 2, 256], f32, tag="mm", name="vps0")
                ps1 = psa.tile([128, 2, 256], f32, tag="mm", name="vps1")
                for k in range(KD):
                    src = srcT[:, k, 128 * tch:128 * tch + 128]
                    for qt in range(4):
                        ps = ps0 if qt < 2 else ps1
                        # start=True clears the WHOLE bank's has_written bits,
                        # so only the first block sharing each bank may clear;
                        # the second block's region was cleared by the same op
                        nc.tensor.matmul(ps[:, qt % 2], src, wvts[qt][:, k],
                                         start=(k == 0 and qt % 2 == 0),
                                         stop=(k == KD - 1))
                for half, ps in ((0, ps0), (1, ps1)):
                    dst = vdst[:, tch, 65 * 8 * half:65 * 8 * half + 520]
                    dst = dst.rearrange("p (h f) -> p h f", f=65)[:, :, 0:64]
                    nc.scalar.copy(
                        dst, ps[:].rearrange("p c (h f) -> p (c h) f", f=64))

        def out_proj_res_ln(wdram, bias_t, onT, resT, g_t, bt_t, dstT):
            xres = a8("xres")
            for ec in range(8):
                wt = wp.tile([128, KD, 128], bf16, tag="w8", name="wot")
                nc.sync.dma_start(wt[:], wdram[ec])
                ps = psa.tile([128, TM], f32, tag="mm", name="ops")
                for j in range(KD):
                    k = (j + ec + 1) % KD
                    nc.tensor.matmul(ps[:], wt[:, k], onT[:, k],
                                     start=(j == 0), stop=(j == KD - 1))
                nc.vector.scalar_tensor_tensor(
                    xres[:, ec], ps[:], bias_t[:, ec:ec + 1], resT[:, ec],
                    OP.add, OP.add)
            ln(xres, g_t, bt_t, dstT)

        # --- main loop over the core's 4 batch elems ----------------------
        for b in range(BPC):
            xT = a8("xT")
            nc.sync.dma_start(xT[:, :, :],
                              xin[b].rearrange("k p t -> p k t"))

            # A: SA projections
            qT = a8("qT")
            kT = a8("kT")
            qk_proj(wqk_sa, bqk_sa_t, (xT, xT), qT, kT,
                    list(range(8, 16)) + list(range(8)))
            v = actp.tile([128, TCH, H * 65], bf16, tag="a8", name="vT")
            v_proj(wv_sa, xT, v)

            # B: SA attention
            onT = a8("onT")
            attention_sa(qT, kT, v, onT)

            # B2: CA k/v projections from audio — independent of LN1, emitted
            # here so they fill the PE while the SA attention tail drains
            aT = a8("aT")
            nc.sync.dma_start(aT[:, :, :],
                              ain[b].rearrange("k p t -> p k t"))
            qT2 = a8("qT2")
            kT2 = a8("kT2")
            qk_proj(wqk_ca, bqk_ca_t, (None, aT), qT2, kT2, list(range(8, 16)))
            v2 = actp.tile([128, TCH, H * 65], bf16, tag="a8", name="v2T")
            v_proj(wv_ca, aT, v2)

            # C: SA out-proj + residual + LN1
            x1T = a8("x1T")
            out_proj_res_ln(wo_sa, bo_sa_t, onT, xT, n1g_t, n1b_t, x1T)

            # D: CA q projection from x1
            qk_proj(wqk_ca, bqk_ca_t, (x1T, aT), qT2, kT2, list(range(8)))

            # E: CA attention (banded, bias via identity matmul)
            onT2 = a8("onT2")
            attention_ca(qT2, kT2, v2, onT2)

            # F: CA out-proj (tanh(gate) folded on host) + residual + LNc
            x2T = a8("x2T")
            out_proj_res_ln(wo_ca, bo_ca_t, onT2, x1T, ncg_t, ncb_t, x2T)

            # G: lin1 + gelu -> hT quarters (bf16)
            hq = [a8(f"hq{i}") for i in range(4)]
            for fc in range(KF):
                wt = wp.tile([128, KD, 128], bf16, tag="w8", name="w1t")
                nc.sync.dma_start(wt[:], w1[fc])
                ps = psa.tile([128, TM], f32, tag="mm", name="hps")
                for j in range(KD):
                    k = (j + fc) % KD
                    nc.tensor.matmul(ps[:], wt[:, k], x2T[:, k],
                                     start=(j == 0), stop=(j == KD - 1))
                nc.scalar.activation(hq[fc // 8][:, fc % 8], ps[:], AF.Gelu,
                                     bias=b1_t[:, fc:fc + 1])

            # H: lin2 + residual + LN2 -> out
            xres2 = a8("xres2")
            for ec in range(8):
                ps = psa.tile([128, TM], f32, tag="mm", name="fps")
                for half in range(2):
                    w2t = w2p.tile([128, KF // 2, 128], bf16, tag="w2",
                                   name="w2t")
                    nc.sync.dma_start(w2t[:], w2[ec, half])
                    for fo in range(KF // 2):
                        fg = half * (KF // 2) + fo
                        nc.tensor.matmul(ps[:], w2t[:, fo],
                                         hq[fg // 8][:, fg % 8, :],
                                         start=(fg == 0), stop=(fg == KF - 1))
                nc.vector.scalar_tensor_tensor(
                    xres2[:, ec], ps[:], b2_t[:, ec:ec + 1], x2T[:, ec],
                    OP.add, OP.add)
            outT = a8("outT")
            ln(xres2, n2g_t, n2b_t, outT)
            nc.sync.dma_start(out[b].rearrange("k p t -> p k t"),
                              outT[:, :, :])

    return nc


def _get_module(ln_affine_identity=True):
    key = ("nc", ln_affine_identity)
    if key not in _CACHE:
        _install_patches()
        _CACHE[key] = _build_module(ln_affine_identity)
    return _CACHE[key]


# ---------------------------------------------------------------------------
# Host-side prep + execution
# ---------------------------------------------------------------------------
def _beat_bias(beats):
    beats = np.asarray(beats).astype(np.int64).ravel()
    bias = np.zeros(TA, np.float32)
    l_idx = np.where(beats > 0, beats - 1, 0)
    l_val = np.where(beats > 0, BW * 0.5, 0.0).astype(np.float32)
    r_idx = np.where(beats < TA - 1, beats + 1, TA - 1)
    r_val = np.where(beats < TA - 1, BW * 0.5, 0.0).astype(np.float32)
    np.maximum.at(bias, l_idx, l_val)
    np.maximum.at(bias, r_idx, r_val)
    np.maximum.at(bias, beats, np.float32(BW))
    return bias


def _temporal_bias():
    scale = (TA - 1) / (TM - 1)
    audio_pos = np.arange(TM, dtype=np.float32) * scale
    diff = audio_pos[:, None] - np.arange(TA, dtype=np.float32)[None, :]
    return (-(diff ** 2) / (2.0 * SIGMA ** 2)).astype(np.float32)


def _chunk_w(w, n_out_chunks, n_in_chunks, dt):
    # w: [E, Dk] row-major -> [ec, p(in), kc, j(out)]
    E, Dk = w.shape
    return np.ascontiguousarray(
        w.reshape(n_out_chunks, E // n_out_chunks, n_in_chunks, Dk // n_in_chunks)
        .transpose(0, 3, 2, 1).astype(dt))


def _pp(vec):
    # [n*128] -> [128, n] per-partition layout
    v = np.asarray(vec, np.float32).reshape(-1, 128)
    return np.ascontiguousarray(v.T)


def kernel(**inputs):
    import ml_dtypes
    from concourse.bass_utils import run_bass_kernel_spmd

    bf16 = ml_dtypes.bfloat16
    f32 = np.float32

    src = np.asarray(inputs["src"], f32)
    audio = np.asarray(inputs["audio_memory"], f32)
    beats = inputs["beat_frames"]

    ln_id = not (
        np.any(np.asarray(inputs["n1_b"])) or np.any(np.asarray(inputs["nc_b"]))
        or np.any(np.asarray(inputs["n2_b"]))
        or np.any(np.asarray(inputs["n1_g"]) != 1.0)
        or np.any(np.asarray(inputs["nc_g"]) != 1.0)
        or np.any(np.asarray(inputs["n2_g"]) != 1.0))
    nc = _get_module(ln_id)

    # feature-major: [B, KD, 128, T]
    xin_all = np.ascontiguousarray(
        src.transpose(1, 2, 0).reshape(B, KD, 128, TM).astype(bf16))
    ain_all = np.ascontiguousarray(
        audio.transpose(1, 2, 0).reshape(B, KD, 128, TA).astype(bf16))

    sa_in_w = np.asarray(inputs["sa_in_w"], f32)
    sa_in_b = np.asarray(inputs["sa_in_b"], f32)
    sa_out_w = np.asarray(inputs["sa_out_w"], f32)
    sa_out_b = np.asarray(inputs["sa_out_b"], f32)
    ca_in_w = np.asarray(inputs["ca_in_w"], f32)
    ca_in_b = np.asarray(inputs["ca_in_b"], f32)
    ca_out_w = np.asarray(inputs["ca_out_w"], f32)
    ca_out_b = np.asarray(inputs["ca_out_b"], f32)
    gate = float(np.asarray(inputs["gate"]))
    tg = float(np.tanh(gate))

    # SA: fold 1/8 score scale into q weights+bias; v-bias into out-proj bias.
    wqk_sa_eff = np.concatenate([sa_in_w[:D] / 8.0, sa_in_w[D:2 * D]], axis=0)
    bqk_sa_eff = np.concatenate([sa_in_b[:D] / 8.0, sa_in_b[D:2 * D]])
    bo_sa_eff = sa_out_b + sa_out_w @ sa_in_b[2 * D:]
    # CA: same folds + tanh(gate) into out-proj weights/bias.
    wqk_ca_eff = np.concatenate([ca_in_w[:D] / 8.0, ca_in_w[D:2 * D]], axis=0)
    bqk_ca_eff = np.concatenate([ca_in_b[:D] / 8.0, ca_in_b[D:2 * D]])
    wo_ca_eff = tg * ca_out_w
    bo_ca_eff = tg * (ca_out_b + ca_out_w @ ca_in_b[2 * D:])

    bias = _temporal_bias() + _beat_bias(beats)[None, :]  # [tq, tk]
    bT = bias.T  # [tk, tq]
    biasb = np.full((4, 128, BBW), -1e9, f32)
    for c in range(4):
        lo, w = TQR[c]
        biasb[c, :, 0:w] = bT[128 * c:128 * c + 128, lo:lo + w]

    w2_arr = _chunk_w(np.asarray(inputs["lin2_w"], f32), 8, KF, bf16)
    weights = {
        "wqk_sa": _chunk_w(wqk_sa_eff, 16, KD, bf16),
        "wv_sa": _chunk_w(sa_in_w[2 * D:], 4, KD, bf16),
        "wo_sa": _chunk_w(sa_out_w, 8, KD, bf16),
        "wqk_ca": _chunk_w(wqk_ca_eff, 16, KD, bf16),
        "wv_ca": _chunk_w(ca_in_w[2 * D:], 4, KD, bf16),
        "wo_ca": _chunk_w(wo_ca_eff, 8, KD, bf16),
        "w1": _chunk_w(np.asarray(inputs["lin1_w"], f32), KF, KD, bf16),
        "w2": np.ascontiguousarray(
            w2_arr.reshape(8, 128, 2, KF // 2, 128).transpose(0, 2, 1, 3, 4)),
        "ppb": np.ascontiguousarray(np.concatenate([
            _pp(bqk_sa_eff), _pp(bo_sa_eff), _pp(bqk_ca_eff), _pp(bo_ca_eff),
            _pp(np.asarray(inputs["lin1_b"], f32)),
            _pp(np.asarray(inputs["lin2_b"], f32)),
            _pp(np.asarray(inputs["n1_g"], f32)),
            _pp(np.asarray(inputs["n1_b"], f32)),
            _pp(np.asarray(inputs["nc_g"], f32)),
            _pp(np.asarray(inputs["nc_b"], f32)),
            _pp(np.asarray(inputs["n2_g"], f32)),
            _pp(np.asarray(inputs["n2_b"], f32)),
        ], axis=1)),
        "biasb": biasb,
        "ident": np.eye(128, dtype=f32),
        "onescol": np.ones((128, 1), f32),
        "onesrow": np.ones((1, 128), f32),
    }

    in_maps = []
    for c in range(NCORES):
        m = dict(weights)
        m["xin"] = np.ascontiguousarray(xin_all[BPC * c:BPC * (c + 1)])
        m["ain"] = np.ascontiguousarray(ain_all[BPC * c:BPC * (c + 1)])
        in_maps.append(m)

    res = run_bass_kernel_spmd(nc, in_maps, core_ids=list(range(NCORES)))
    outs = [r["out"] for r in res.results]  # each [BPC, KD, 128, TM] bf16
    full = np.concatenate(outs, axis=0).astype(np.float32)
    return np.ascontiguousarray(
        full.reshape(B, D, TM).transpose(2, 0, 1))
